# revision 3
# baseline (speedup 1.0000x reference)
"""Differentiable SVM (hinge-loss GD + linear predict) on 8 Trainium2 cores.

Strategy (v2 — gradient AllReduce):
  - Support rows sharded 512/core; W replicated on every core (f32 master in
    embed-major k-tile layout + bf16 matmul copy). Queries sharded 2048/core.
  - Per GD iteration each core computes the FULL local-partial gradient
    gl^T X_c (classes-major, 16 matmuls of free-512) and the bias gradient;
    two AllReduces (embed halves, bf16 blobs ~262/294KB) sum the partials.
  - Collective blobs are contiguous [128, F] rows: packs/unpacks are plain
    128-descriptor DMAs. The post-AR classes-major -> embed-major transpose
    is a single dma_start_transpose (xbar) per half, straight into the
    k-tile layout the scores/query matmuls consume. No per-iteration PE
    transposes except the 4 scores m-tiles feeding the hinge.
  - Iteration 0 is closed-form (W=0 => G0 = 1-128*oh), so W_1 is computed
    host-side and shipped; the device runs iterations 1..14.
  - All inputs are host-pre-tiled to [128, F] SBUF layouts so every initial
    DMA is 128 fat descriptors.
  - Query phase: out^T = W^T Q^T, k-major over 4 chunks of 512; Q^T slices
    prefetched one per iteration during the fit.
"""
import os

import numpy as np
import ml_dtypes

import concourse.bass as bass
import concourse.bacc as bacc
import concourse.masks as masks
import concourse.mybir as mybir
import concourse.tile as tile
from concourse.bass_utils import run_bass_kernel_spmd

BF16 = ml_dtypes.bfloat16
F32 = mybir.dt.float32
BF = mybir.dt.bfloat16
ALU = mybir.AluOpType

NCORES = 8
N_SUP = 4096
D = 2048
KCLS = 128
N_Q = 16384
SROWS = N_SUP // NCORES     # 512 support rows / core (4 row k-tiles)
QROWS = N_Q // NCORES       # 2048 query rows / core (4 chunks of 512)
ITERS = 15                  # total GD iterations; it 0 is host-side
LR = np.float32(0.01)
CREG = np.float32(1.0)
NK = np.float32(N_SUP * KCLS)
DECAY = float(np.float32(1.0) - LR * CREG)   # 0.99
LRNK = float(LR / NK)
KT = D // 128               # 16 embed k-tiles
RT = SROWS // 128           # 4 support-row k-tiles
HW_ = D // 2                # 1024 embed cols per AR half
BW = HW_ + KCLS             # half-B blob width (bias col block appended)
GROUP = [list(range(NCORES))]


def build():
    nc = bacc.Bacc("TRN2", target_bir_lowering=False, debug=False,
                   num_devices=NCORES)

    xst = nc.dram_tensor("xst", [128, KT * SROWS], BF, kind="ExternalInput")
    xloc = nc.dram_tensor("xloc", [128, RT * D], BF, kind="ExternalInput")
    oht = nc.dram_tensor("oht", [128, RT * KCLS], BF, kind="ExternalInput")
    qtt = nc.dram_tensor("qtt", [128, KT * QROWS], BF, kind="ExternalInput")
    w1f = nc.dram_tensor("w1f", [128, KT * KCLS], F32, kind="ExternalInput")
    w1bf = nc.dram_tensor("w1bf", [128, KT * KCLS], BF, kind="ExternalInput")
    w1b = nc.dram_tensor("w1b", [1, KCLS], F32, kind="ExternalInput")
    w1bb = nc.dram_tensor("w1bb", [1, KCLS], BF, kind="ExternalInput")
    outT = nc.dram_tensor("outT", [KCLS, QROWS], F32, kind="ExternalOutput")

    with tile.TileContext(nc) as tc:
        with (
            tc.tile_pool(name="static", bufs=1) as st,
            tc.tile_pool(name="dram", bufs=1, space="DRAM") as dram,
            tc.tile_pool(name="small", bufs=8) as sm,
            tc.tile_pool(name="scratch", bufs=4) as scr,
        ):
            # ---- static SBUF tensors ----
            xst_sb = st.tile([128, KT * SROWS], BF)     # X_c^T k-tiles
            xloc_sb = st.tile([128, RT * D], BF)        # X_c row k-tiles
            oh_sb = st.tile([128, RT * KCLS], BF)       # one-hot row k-tiles
            qt_sb = st.tile([128, KT * QROWS], BF)      # Q_c^T (prefetched)
            w_f32 = st.tile([128, KT * KCLS], F32)      # W master, embed-major
            w_sb = st.tile([128, KT * KCLS], BF)        # W bf16, embed-major
            wb_f32 = st.tile([1, KCLS], F32)            # bias row master
            wb_sb = st.tile([1, KCLS], BF)              # bias row bf16
            gl_sb = st.tile([128, RT * KCLS], BF)       # -NK*G local, row tiles
            sT_sb = st.tile([128, SROWS], BF)           # scores^T bf16
            gpk0 = st.tile([128, HW_], BF)              # AR pack half A
            gpk1 = st.tile([128, BW], BF)               # AR pack half B + bias
            gsum0 = st.tile([128, HW_], BF)             # AR result T, half A
            gsum1 = st.tile([128, BW], BF)              # AR result T, half B
            ones_c = st.tile([128, 1], BF)              # ones col (gradb rhs)
            ones_r = st.tile([1, SROWS], BF)            # ones row (bias rhs)
            id_bf = st.tile([128, 128], BF)

            nc.vector.memset(gpk1[:, HW_:BW], 0.0)  # bias block cols 1..127
            nc.vector.memset(ones_c[:], 1.0)
            nc.vector.memset(ones_r[:], 1.0)
            masks.make_identity(nc, id_bf[:])

            # ---- initial loads (host pre-tiled: 128 fat descriptors each) ----
            nc.sync.dma_start(w_sb[:], w1bf[:])
            nc.sync.dma_start(wb_sb[:], w1bb[:])
            for lo, hi in ((0, 4), (4, 8), (8, 12), (12, 16)):
                nc.sync.dma_start(xst_sb[:, lo * SROWS:hi * SROWS],
                                  xst[:, lo * SROWS:hi * SROWS])
            nc.sync.dma_start(oh_sb[:], oht[:])
            for lo, hi in ((0, 2), (2, 4)):
                nc.scalar.dma_start(xloc_sb[:, lo * D:hi * D],
                                    xloc[:, lo * D:hi * D])
            nc.scalar.dma_start(w_f32[:], w1f[:])
            nc.scalar.dma_start(wb_f32[:], w1b[:])

            with (
                tc.tile_pool(name="ps_sc", bufs=1, space="PSUM") as ps_sc,
                tc.tile_pool(name="ps_m", bufs=2, space="PSUM") as ps_m,
                tc.tile_pool(name="ps_g", bufs=2, space="PSUM") as ps_g,
                tc.tile_pool(name="ps_b", bufs=1, space="PSUM") as ps_b,
            ):
                for it in range(1, ITERS):
                    # ---- scores^T = W^T X_c^T + b : [classes, 512] ----
                    psT = ps_sc.tile([128, SROWS], F32, tag="psT",
                                     name=f"psT_{it}")
                    for k in range(KT):
                        nc.tensor.matmul(
                            psT[:],
                            w_sb[:, k * KCLS:(k + 1) * KCLS],
                            xst_sb[:, k * SROWS:(k + 1) * SROWS],
                            start=(k == 0), stop=False)
                    nc.tensor.matmul(psT[:], wb_sb[:], ones_r[:],
                                     start=False, stop=True)

                    # ---- hinge: gl = oh*ssum - stepb = -NK*G ----
                    for m in range(RT):
                        nc.vector.tensor_copy(
                            sT_sb[:, m * 128:(m + 1) * 128],
                            psT[:, m * 128:(m + 1) * 128])
                        psm = ps_m.tile([128, 128], BF, tag="psm",
                                        name=f"psm_{it}_{m}")
                        nc.tensor.transpose(
                            psm[:], sT_sb[:, m * 128:(m + 1) * 128],
                            id_bf[:])
                        ohm = oh_sb[:, m * KCLS:(m + 1) * KCLS]
                        junk = scr.tile([128, KCLS], BF, tag="junk",
                                        name=f"junk_{it}_{m}")
                        corr = sm.tile([128, 1], F32, tag="corr",
                                       name=f"corr_{it}_{m}")
                        ssum = sm.tile([128, 1], F32, tag="ssum",
                                       name=f"ssum_{it}_{m}")
                        stepb = scr.tile([128, KCLS], BF, tag="stepb",
                                         name=f"stepb_{it}_{m}")
                        nc.vector.scalar_tensor_tensor(
                            out=junk[:], in0=psm[:], scalar=1.0,
                            in1=ohm, op0=ALU.mult, op1=ALU.mult,
                            accum_out=corr[:])
                        nc.vector.tensor_scalar(
                            out=stepb[:], in0=psm[:],
                            scalar1=corr[:], scalar2=-1.0,
                            op0=ALU.subtract, op1=ALU.is_gt)
                        nc.vector.tensor_reduce(
                            out=ssum[:], in_=stepb[:],
                            axis=mybir.AxisListType.X, op=ALU.add)
                        nc.vector.scalar_tensor_tensor(
                            out=gl_sb[:, m * KCLS:(m + 1) * KCLS],
                            in0=ohm, scalar=ssum[:], in1=stepb[:],
                            op0=ALU.mult, op1=ALU.subtract)

                    # ---- -NK*gradT chunks + bias grad; pack halves ----
                    gin0 = dram.tile([128, HW_], BF, tag=f"gi0_{it}",
                                     name=f"gi0_{it}")
                    gin1 = dram.tile([128, BW], BF, tag=f"gi1_{it}",
                                     name=f"gi1_{it}")
                    gout0 = dram.tile([128, HW_], BF, addr_space="Shared",
                                      tag=f"go0_{it}", name=f"go0_{it}")
                    gout1 = dram.tile([128, BW], BF, addr_space="Shared",
                                      tag=f"go1_{it}", name=f"go1_{it}")
                    psgb = ps_b.tile([128, 1], F32, tag="psgb",
                                     name=f"psgb_{it}")
                    for c in range(4):
                        psg = ps_g.tile([128, 512], F32, tag="psg",
                                        name=f"psg_{it}_{c}")
                        for k in range(RT):
                            nc.tensor.matmul(
                                psg[:],
                                gl_sb[:, k * KCLS:(k + 1) * KCLS],
                                xloc_sb[:, k * D + c * 512:
                                        k * D + (c + 1) * 512],
                                start=(k == 0), stop=(k == RT - 1))
                        if c == 1:
                            for k in range(RT):
                                nc.tensor.matmul(
                                    psgb[:],
                                    gl_sb[:, k * KCLS:(k + 1) * KCLS],
                                    ones_c[:],
                                    start=(k == 0), stop=(k == RT - 1))
                        gpk = gpk0 if c < 2 else gpk1
                        nc.scalar.copy(
                            gpk[:, (c % 2) * 512:(c % 2) * 512 + 512],
                            psg[:])
                        if c == 1:
                            nc.scalar.copy(gpk1[:, HW_:HW_ + 1], psgb[:])
                            nc.sync.dma_start(gin0[:], gpk0[:])
                            nc.gpsimd.collective_compute(
                                "AllReduce", ALU.add, replica_groups=GROUP,
                                ins=[gin0[:]], outs=[gout0[:]])
                        if c == 3:
                            nc.sync.dma_start(gin1[:], gpk1[:])
                            nc.gpsimd.collective_compute(
                                "AllReduce", ALU.add, replica_groups=GROUP,
                                ins=[gin1[:]], outs=[gout1[:]])

                    # ---- unpack+transpose (xbar), update masters, cast ----
                    nc.sync.dma_start_transpose(
                        gsum0[:].rearrange("p (k c) -> p k c", c=128),
                        gout0[:])
                    nc.sync.dma_start_transpose(
                        gsum1[:].rearrange("p (k c) -> p k c", c=128),
                        gout1[:])
                    for h in range(2):
                        wh = w_f32[:, h * HW_:(h + 1) * HW_]
                        nc.vector.tensor_scalar_mul(wh, wh, DECAY)
                        gs = (gsum0 if h == 0 else gsum1)[:, 0:HW_]
                        nc.vector.scalar_tensor_tensor(
                            out=wh, in0=gs, scalar=LRNK, in1=wh,
                            op0=ALU.mult, op1=ALU.add)
                        nc.vector.tensor_copy(
                            w_sb[:, h * HW_:(h + 1) * HW_], wh)
                    nc.vector.scalar_tensor_tensor(
                        out=wb_f32[:], in0=gsum1[0:1, HW_:HW_ + KCLS],
                        scalar=LRNK, in1=wb_f32[:],
                        op0=ALU.mult, op1=ALU.add)
                    nc.vector.tensor_copy(wb_sb[:], wb_f32[:])

                    # ---- Q^T prefetch: one k-slice per iteration ----
                    for k in range(KT):
                        if k % (ITERS - 1) == it - 1:
                            nc.scalar.dma_start(
                                qt_sb[:, k * QROWS:(k + 1) * QROWS],
                                qtt[:, k * QROWS:(k + 1) * QROWS])

            # ---- query phase: out^T = W^T Q^T + b ----
            with (
                tc.tile_pool(name="qout", bufs=2) as qout,
                tc.tile_pool(name="ps_q", bufs=1, space="PSUM") as ps_q,
            ):
                NCHUNK = QROWS // 512
                pqs = [ps_q.tile([128, 512], F32, tag=f"pq{ch}",
                                 name=f"pq_{ch}") for ch in range(NCHUNK)]
                for k in range(KT):
                    for ch in range(NCHUNK):
                        nc.tensor.matmul(
                            pqs[ch][:],
                            w_sb[:, k * KCLS:(k + 1) * KCLS],
                            qt_sb[:, k * QROWS + ch * 512:
                                  k * QROWS + (ch + 1) * 512],
                            start=(k == 0), stop=False)
                for ch in range(NCHUNK):
                    nc.tensor.matmul(pqs[ch][:], wb_sb[:],
                                     ones_r[:, 0:512],
                                     start=False, stop=True)
                    qo = qout.tile([128, 512], F32, tag="qo",
                                   name=f"qo_{ch}")
                    nc.vector.tensor_copy(qo[:], pqs[ch][:])
                    nc.sync.dma_start(
                        outT[:, ch * 512:(ch + 1) * 512], qo[:])
    nc.compile()
    return nc


def _tile128(a, p=128):
    """[K*p, F] row-major -> [p, K*F] k-tile SBUF layout."""
    k = a.shape[0] // p
    return np.ascontiguousarray(
        a.reshape(k, p, a.shape[1]).transpose(1, 0, 2).reshape(p, -1))


def _prep_inputs(support_embeddings, support_labels, query_embeddings):
    X = np.asarray(support_embeddings, dtype=np.float32)
    labels = np.asarray(support_labels).astype(np.int64)
    Q = np.asarray(query_embeddings, dtype=np.float32)

    oh_full = (labels[:, None] == np.arange(KCLS)[None, :]).astype(np.float32)
    # host iteration 0: W=0 -> G0 = 1 - KCLS*oh (unscaled); W1 = -(LR/NK) Xb^T G0
    g0 = 1.0 - np.float32(KCLS) * oh_full
    gtop = X.T.astype(np.float32) @ g0            # [2048, 128]
    gbias = g0.sum(axis=0, keepdims=True)         # [1, 128]
    w1 = (-LR / NK) * gtop
    w1bias = (-LR / NK) * gbias

    w1f = _tile128(w1.astype(np.float32))
    w1bf = _tile128(w1.astype(BF16))
    w1b = np.ascontiguousarray(w1bias.astype(np.float32))
    w1bb = np.ascontiguousarray(w1bias.astype(BF16))

    in_maps = []
    for c in range(NCORES):
        rs, re = c * SROWS, (c + 1) * SROWS
        qs, qe = c * QROWS, (c + 1) * QROWS
        Xc = X[rs:re]
        in_maps.append({
            "xst": _tile128(np.ascontiguousarray(Xc.T).astype(BF16)),
            "xloc": _tile128(Xc.astype(BF16)),
            "oht": _tile128(oh_full[rs:re].astype(BF16)),
            "qtt": _tile128(np.ascontiguousarray(Q[qs:qe].T).astype(BF16)),
            "w1f": w1f, "w1bf": w1bf, "w1b": w1b, "w1bb": w1bb,
        })
    return in_maps


_NC_CACHE = None


def kernel(support_embeddings, support_labels, query_embeddings,
           n_classes=KCLS, **_):
    global _NC_CACHE
    if _NC_CACHE is None:
        _NC_CACHE = build()
    nc = _NC_CACHE
    in_maps = _prep_inputs(support_embeddings, support_labels,
                           query_embeddings)
    trace = bool(os.environ.get("KERNEL_TRACE"))
    res = run_bass_kernel_spmd(nc, in_maps, core_ids=list(range(NCORES)),
                               trace=trace)
    if trace and res.exec_time_ns is not None:
        print(f"HW exec time: {res.exec_time_ns} ns")
    out = np.concatenate(
        [res.results[c]["outT"].T for c in range(NCORES)], axis=0)
    return np.ascontiguousarray(out.astype(np.float32))
